# revision 14
# baseline (speedup 1.0000x reference)
"""ArcMarginProduct (ArcFace) forward on 8 TRN2 NeuronCores.

out[b, c] = s * cos(theta_bc)         except at c == label[b] where
out[b, c] = s * phi(cos(theta_bc))    (margin epilogue)

Strategy (classification-parallel / Partial-FC):
  - pad C 84281 -> 84992 = 8 * 10624, shard class rows across 8 cores
  - each core computes out_shard^T = [10624, 512] (classes x batch), bf16
  - margin fix touches only 512 scattered elements -> indirect DMA
  - host concatenates shards, drops padding, transposes, casts to f32

Per-core pipeline per 128-class tile:
  DMA w [128,512] f32 -> ACT Square+accum (row sumsq) -> sqrt/recip ->
  DVE tensor_scalar (x 1/||w||, cast bf16) -> DMA xbar transpose ->
  PE 4x matmul (vs s*xn^T bf16) -> DVE PSUM->SBUF copy (bf16) -> DMA out
"""

import math

import numpy as np

B = 512
D = 512
C = 84281
NCORES = 8
CS = 10624          # padded classes per core (83 * 128)
NT = CS // 128      # 83 class tiles per core
REAL = [10536] * 7 + [C - 10536 * 7]   # real class rows per core (<= CS-1)
BASE = [10536 * i for i in range(NCORES)]
PAD_ROW = CS - 1    # always-padding row, scatter dump for out-of-range labels

S_SCALE = 32.0
MARGIN = 0.5
COS_M = math.cos(MARGIN)
SIN_M = math.sin(MARGIN)
TH = math.cos(math.pi - MARGIN)
MM = math.sin(math.pi - MARGIN) * MARGIN

_CACHE = {}


def _build_nc():
    import concourse.tile as tile
    from concourse import bacc, mybir
    from concourse.bass import IndirectOffsetOnAxis
    from contextlib import ExitStack

    f32 = mybir.dt.float32
    bf16 = mybir.dt.bfloat16
    i32 = mybir.dt.int32

    nc = bacc.Bacc("TRN2", target_bir_lowering=False, debug=False, num_devices=NCORES)
    x_ext = nc.declare_dram_parameter("x", [B, D], f32, isOutput=False)
    w_ext = nc.declare_dram_parameter("weight", [CS, D], f32, isOutput=False)
    idx_ext = nc.declare_dram_parameter("idx", [128, 4], i32, isOutput=False)
    soff_ext = nc.declare_dram_parameter("soff", [128, 4], i32, isOutput=False)
    out_ext = nc.declare_dram_parameter("out", [CS, B], bf16, isOutput=True)

    w_view = w_ext[:].rearrange("(t p) d -> p t d", p=128)      # [128, 83, 512]
    x_view = x_ext[:].rearrange("(i p) d -> p i d", p=128)      # [128, 4, 512]
    out_view = out_ext[:].rearrange("(t p) b -> p t b", p=128)  # [128, 83, 512]
    out_flat = out_ext[:].rearrange("r c -> (r c)").unsqueeze(-1)  # [CS*B, 1]

    with tile.TileContext(nc) as tc, ExitStack() as es:
        cpool = es.enter_context(tc.tile_pool(name="consts", bufs=1))
        spool = es.enter_context(tc.tile_pool(name="small", bufs=2))
        wpool = es.enter_context(tc.tile_pool(name="wch", bufs=3))
        wnbpool = es.enter_context(tc.tile_pool(name="wnb", bufs=3))
        outpool = es.enter_context(tc.tile_pool(name="outch", bufs=3))
        wtpool = es.enter_context(tc.tile_pool(name="wt", bufs=4))
        ppool_out = es.enter_context(tc.tile_pool(name="pout", bufs=3, space="PSUM"))

        # ---- x: load, normalize (keep fp32), build s*xn^T bf16 [d, b]
        x_sb = cpool.tile([128, 4, D], f32, tag="x_sb")
        nc.sync.dma_start(out=x_sb[:], in_=x_view)
        scr = spool.tile([128, D], bf16, tag="scr")
        ssx = cpool.tile([128, 4], f32, tag="ssx")
        for i in range(4):
            nc.scalar.activation(
                out=scr[:],
                in_=x_sb[:, i, :],
                func=mybir.ActivationFunctionType.Square,
                accum_out=ssx[:, i : i + 1],
            )
        snx = cpool.tile([128, 4], f32, tag="snx")
        nc.scalar.sqrt(snx[:], ssx[:])
        xinv = cpool.tile([128, 4], f32, tag="xinv")
        nc.vector.reciprocal(xinv[:], snx[:])
        xn = cpool.tile([128, 4, D], f32, tag="xn")
        for i in range(4):
            nc.vector.tensor_scalar_mul(xn[:, i, :], x_sb[:, i, :], xinv[:, i : i + 1])
        # s * xn in bf16, then xbar-transpose to [d_p, k, i, b_in]
        xnb = cpool.tile([128, 4, D], bf16, tag="xnb")
        nc.vector.tensor_scalar_mul(xnb[:], xn[:], S_SCALE)
        xnT = cpool.tile([128, 4, 4, 128], bf16, tag="xnT")  # [dp, k, i, b]
        for i in range(4):
            nc.scalar.dma_start_transpose(xnT[:, :, i, :], xnb[:, i, :])

        # ---- label path: gather w[label], cos at label, phi values
        idx_sb = cpool.tile([128, 4], i32, tag="idx_sb")
        nc.sync.dma_start(out=idx_sb[:], in_=idx_ext[:])
        soff_sb = cpool.tile([128, 4], i32, tag="soff_sb")
        nc.sync.dma_start(out=soff_sb[:], in_=soff_ext[:])
        wlab = cpool.tile([128, 4, D], f32, tag="wlab")
        for i in range(4):
            nc.gpsimd.indirect_dma_start(
                out=wlab[:, i, :],
                out_offset=None,
                in_=w_ext[:],
                in_offset=IndirectOffsetOnAxis(ap=idx_sb[:, i : i + 1], axis=0),
            )
        ssl = cpool.tile([128, 4], f32, tag="ssl")
        dot = cpool.tile([128, 4], f32, tag="dot")
        prod = cpool.tile([128, D], f32, tag="prod")
        for i in range(4):
            nc.scalar.activation(
                out=scr[:],
                in_=wlab[:, i, :],
                func=mybir.ActivationFunctionType.Square,
                accum_out=ssl[:, i : i + 1],
            )
        for i in range(4):
            nc.vector.tensor_tensor(
                prod[:], xn[:, i, :], wlab[:, i, :], op=mybir.AluOpType.mult
            )
            nc.vector.reduce_sum(
                dot[:, i : i + 1], prod[:], axis=mybir.AxisListType.X
            )
        snl = cpool.tile([128, 4], f32, tag="snl")
        nc.scalar.sqrt(snl[:], ssl[:])
        slinv = cpool.tile([128, 4], f32, tag="slinv")
        nc.vector.reciprocal(slinv[:], snl[:])
        cosl = cpool.tile([128, 4], f32, tag="cosl")
        nc.vector.tensor_tensor(cosl[:], dot[:], slinv[:], op=mybir.AluOpType.mult)
        # sine = sqrt(max(0, 1 - cos^2))
        sq = cpool.tile([128, 4], f32, tag="sq")
        nc.vector.tensor_tensor(sq[:], cosl[:], cosl[:], op=mybir.AluOpType.mult)
        sin2 = cpool.tile([128, 4], f32, tag="sin2")
        nc.vector.tensor_scalar(
            sin2[:], sq[:], -1.0, 1.0,
            op0=mybir.AluOpType.mult, op1=mybir.AluOpType.add,
        )
        nc.vector.tensor_scalar_max(sin2[:], sin2[:], 0.0)
        sine = cpool.tile([128, 4], f32, tag="sine")
        nc.scalar.sqrt(sine[:], sin2[:])
        # phi = cos*cos_m - sine*sin_m
        t1 = cpool.tile([128, 4], f32, tag="t1")
        nc.vector.tensor_scalar_mul(t1[:], cosl[:], COS_M)
        t2 = cpool.tile([128, 4], f32, tag="t2")
        nc.vector.tensor_scalar_mul(t2[:], sine[:], SIN_M)
        phi = cpool.tile([128, 4], f32, tag="phi")
        nc.vector.tensor_tensor(phi[:], t1[:], t2[:], op=mybir.AluOpType.subtract)
        # where(cos > th, phi, cos - mm), then * s
        alt = cpool.tile([128, 4], f32, tag="alt")
        nc.vector.tensor_scalar_sub(alt[:], cosl[:], MM)
        mask = cpool.tile([128, 4], mybir.dt.uint8, tag="mask")
        nc.vector.tensor_scalar(
            mask[:], cosl[:], TH, None, op0=mybir.AluOpType.is_gt,
        )
        phif = cpool.tile([128, 4], f32, tag="phif")
        nc.vector.tensor_copy(phif[:], alt[:])
        nc.vector.copy_predicated(phif[:], mask[:], phi[:])
        val = cpool.tile([128, 4], bf16, tag="val")
        nc.vector.tensor_scalar_mul(val[:], phif[:], S_SCALE)

        # ---- main loop over class-tile chunks
        chunk_sizes = [4] * 20 + [3]
        c0 = 0
        for n in chunk_sizes:
            wch = wpool.tile([128, n, D], f32, tag="wch")
            nc.sync.dma_start(out=wch[:], in_=w_view[:, c0 : c0 + n, :])
            ssw = spool.tile([128, 4], f32, tag="ssw")
            scrw = spool.tile([128, D], bf16, tag="scrw")
            for j in range(n):
                nc.scalar.activation(
                    out=scrw[:],
                    in_=wch[:, j, :],
                    func=mybir.ActivationFunctionType.Square,
                    accum_out=ssw[:, j : j + 1],
                )
            wn = spool.tile([128, 4], f32, tag="wn")
            nc.scalar.sqrt(wn[:, :n], ssw[:, :n])
            winv = spool.tile([128, 4], f32, tag="winv")
            nc.vector.reciprocal(winv[:, :n], wn[:, :n])
            wnb = wnbpool.tile([128, n, D], bf16, tag="wnb")
            for j in range(n):
                nc.vector.tensor_scalar_mul(
                    wnb[:, j, :], wch[:, j, :], winv[:, j : j + 1]
                )
            outch = outpool.tile([128, n, B], bf16, tag="outch")
            for g0 in range(0, n, 2):
                ng = min(2, n - g0)
                po = ppool_out.tile([128, 2 * B], f32, name="po")
                for jj in range(ng):
                    j = g0 + jj
                    wT = wtpool.tile([128, 4, 128], bf16, tag="wT")
                    nc.scalar.dma_start_transpose(wT[:], wnb[:, j, :])
                    for k in range(4):
                        nc.tensor.matmul(
                            po[:, jj * B : (jj + 1) * B],
                            lhsT=wT[:, k, :],
                            rhs=xnT[:, k, :, :],
                            start=(k == 0),
                            stop=(k == 3),
                        )
                nc.vector.tensor_copy(
                    outch[:, g0 : g0 + ng, :], po[:, : ng * B]
                )
            nc.sync.dma_start(out=out_view[:, c0 : c0 + n, :], in_=outch[:])
            c0 += n

        # ---- scatter the 512 margin fixups into out
        for i in range(4):
            nc.gpsimd.indirect_dma_start(
                out=out_flat,
                out_offset=IndirectOffsetOnAxis(ap=soff_sb[:, i : i + 1], axis=0),
                in_=val[:, i : i + 1],
                in_offset=None,
            )

    nc.finalize()
    return nc


def _get_nc():
    if "nc" not in _CACHE:
        _CACHE["nc"] = _build_nc()
    return _CACHE["nc"]


def make_in_maps(x, weight, label):
    x = np.asarray(x, dtype=np.float32)
    weight = np.asarray(weight, dtype=np.float32)
    label = np.asarray(label).astype(np.int64)
    in_maps = []
    for i in range(NCORES):
        a, r = BASE[i], REAL[i]
        wshard = np.ones((CS, D), dtype=np.float32)
        wshard[:r] = weight[a : a + r]
        loc = label - a
        in_range = (loc >= 0) & (loc < r)
        idx = np.where(in_range, loc, PAD_ROW).astype(np.int32)
        b = np.arange(B, dtype=np.int64)
        soff = (idx.astype(np.int64) * B + b).astype(np.int32)
        # device layout [128, 4]: column i holds batch rows i*128..i*128+127
        idx_dev = np.ascontiguousarray(idx.reshape(4, 128).T)
        soff_dev = np.ascontiguousarray(soff.reshape(4, 128).T)
        in_maps.append({"x": x, "weight": wshard, "idx": idx_dev, "soff": soff_dev})
    return in_maps


def assemble(results):
    shards = [np.asarray(results[i]["out"])[: REAL[i]] for i in range(NCORES)]
    full_t = np.concatenate(shards, axis=0).astype(np.float32)  # [C, B]
    return np.ascontiguousarray(full_t.T)


def kernel(x, weight, label):
    from concourse.bass_utils import run_bass_kernel_spmd

    nc = _get_nc()
    in_maps = make_in_maps(x, weight, label)
    res = run_bass_kernel_spmd(nc, in_maps, list(range(NCORES)))
    return assemble(res.results)


# revision 19
# speedup vs baseline: 1.7098x; 1.7098x over previous
"""ArcMarginProduct (ArcFace) forward on 8 TRN2 NeuronCores.

out[b, c] = s * cos(theta_bc)         except at c == label[b] where
out[b, c] = s * phi(cos(theta_bc))    (margin epilogue)

Strategy (classification-parallel / Partial-FC):
  - pad C 84281 -> 84992 = 8 * 10624, shard class rows across 8 cores
  - each core computes out_shard^T = [10624, 512] (classes x batch), bf16
  - margin fix touches only 512 scattered elements -> indirect DMA
  - host concatenates shards, drops padding, transposes, casts to f32

Per-core pipeline per 128-class tile:
  DMA w [128,512] f32 -> ACT Square+accum (row sumsq) -> sqrt/recip ->
  DVE tensor_scalar (x 1/||w||, cast bf16) -> DMA xbar transpose ->
  PE 4x matmul (vs s*xn^T bf16) -> DVE PSUM->SBUF copy (bf16) -> DMA out
"""

import math

import numpy as np

B = 512
D = 512
C = 84281
NCORES = 8
CS = 10624          # padded classes per core (83 * 128)
NT = CS // 128      # 83 class tiles per core
REAL = [10536] * 7 + [C - 10536 * 7]   # real class rows per core (<= CS-1)
BASE = [10536 * i for i in range(NCORES)]
PAD_ROW = CS - 1    # always-padding row, scatter dump for out-of-range labels

S_SCALE = 32.0
MARGIN = 0.5
COS_M = math.cos(MARGIN)
SIN_M = math.sin(MARGIN)
TH = math.cos(math.pi - MARGIN)
MM = math.sin(math.pi - MARGIN) * MARGIN

_CACHE = {}


def _build_nc():
    import concourse.tile as tile
    from concourse import bacc, mybir
    from concourse.bass import IndirectOffsetOnAxis
    from concourse.masks import make_identity
    from contextlib import ExitStack

    f32 = mybir.dt.float32
    bf16 = mybir.dt.bfloat16
    i32 = mybir.dt.int32

    nc = bacc.Bacc("TRN2", target_bir_lowering=False, debug=False, num_devices=NCORES)
    x_ext = nc.declare_dram_parameter("x", [B, D], f32, isOutput=False)
    w_ext = nc.declare_dram_parameter("weight", [CS, D], f32, isOutput=False)
    idx_ext = nc.declare_dram_parameter("idx", [128, 4], i32, isOutput=False)
    soff_ext = nc.declare_dram_parameter("soff", [128, 4], i32, isOutput=False)
    out_ext = nc.declare_dram_parameter("out", [CS, B], bf16, isOutput=True)

    w_view = w_ext[:].rearrange("(t p) d -> p t d", p=128)      # [128, 83, 512]
    x_view = x_ext[:].rearrange("(i p) d -> p i d", p=128)      # [128, 4, 512]
    out_view = out_ext[:].rearrange("(t p) b -> p t b", p=128)  # [128, 83, 512]
    out_flat = out_ext[:].rearrange("r c -> (r c)").unsqueeze(-1)  # [CS*B, 1]

    with tile.TileContext(nc) as tc, ExitStack() as es:
        cpool = es.enter_context(tc.tile_pool(name="consts", bufs=1))
        spool = es.enter_context(tc.tile_pool(name="small", bufs=2))
        wpool = es.enter_context(tc.tile_pool(name="wch", bufs=3))
        wnbpool = es.enter_context(tc.tile_pool(name="wnb", bufs=3))
        outpool = es.enter_context(tc.tile_pool(name="outch", bufs=3))
        wtpool = es.enter_context(tc.tile_pool(name="wt", bufs=4))
        ppool_out = es.enter_context(tc.tile_pool(name="pout", bufs=3, space="PSUM"))
        ppool_wt = es.enter_context(tc.tile_pool(name="pwt", bufs=2, space="PSUM"))

        ident = cpool.tile([128, 128], f32, tag="ident")
        make_identity(nc, ident[:])
        ident_bf = cpool.tile([128, 128], bf16, tag="ident_bf")
        nc.vector.tensor_copy(ident_bf[:], ident[:])

        # ---- x: load, normalize (keep fp32), build s*xn^T bf16 [d, b]
        x_sb = cpool.tile([128, 4, D], f32, tag="x_sb")
        nc.sync.dma_start(out=x_sb[:], in_=x_view)
        scr = spool.tile([128, D], bf16, tag="scr")
        ssx = cpool.tile([128, 4], f32, tag="ssx")
        for i in range(4):
            nc.scalar.activation(
                out=scr[:],
                in_=x_sb[:, i, :],
                func=mybir.ActivationFunctionType.Square,
                accum_out=ssx[:, i : i + 1],
            )
        snx = cpool.tile([128, 4], f32, tag="snx")
        nc.scalar.sqrt(snx[:], ssx[:])
        xinv = cpool.tile([128, 4], f32, tag="xinv")
        nc.vector.reciprocal(xinv[:], snx[:])
        xn = cpool.tile([128, 4, D], f32, tag="xn")
        for i in range(4):
            nc.vector.tensor_scalar_mul(xn[:, i, :], x_sb[:, i, :], xinv[:, i : i + 1])
        # s * xn in bf16, then xbar-transpose to [d_p, k, i, b_in] and
        # repack to contiguous [d_p, b] per k (contiguous rhs keeps the
        # matmul moving-operand stream at full rate)
        xnb = cpool.tile([128, 4, D], bf16, tag="xnb")
        nc.vector.tensor_scalar_mul(xnb[:], xn[:], S_SCALE)
        xnT_s = cpool.tile([128, 4, 4, 128], bf16, tag="xnT_s")  # [dp, k, i, b]
        for i in range(4):
            nc.scalar.dma_start_transpose(xnT_s[:, :, i, :], xnb[:, i, :])
        xnT = [
            cpool.tile([128, B], bf16, tag=f"xnT{k}", name=f"xnT{k}")
            for k in range(4)
        ]
        for k in range(4):
            nc.vector.tensor_copy(xnT[k][:], xnT_s[:, k, :, :])

        # ---- label path: gather w[label], cos at label, phi values
        idx_sb = cpool.tile([128, 4], i32, tag="idx_sb")
        nc.sync.dma_start(out=idx_sb[:], in_=idx_ext[:])
        soff_sb = cpool.tile([128, 4], i32, tag="soff_sb")
        nc.sync.dma_start(out=soff_sb[:], in_=soff_ext[:])
        wlab = cpool.tile([128, 4, D], f32, tag="wlab")
        for i in range(4):
            nc.gpsimd.indirect_dma_start(
                out=wlab[:, i, :],
                out_offset=None,
                in_=w_ext[:],
                in_offset=IndirectOffsetOnAxis(ap=idx_sb[:, i : i + 1], axis=0),
            )
        ssl = cpool.tile([128, 4], f32, tag="ssl")
        dot = cpool.tile([128, 4], f32, tag="dot")
        prod = cpool.tile([128, D], f32, tag="prod")
        for i in range(4):
            nc.scalar.activation(
                out=scr[:],
                in_=wlab[:, i, :],
                func=mybir.ActivationFunctionType.Square,
                accum_out=ssl[:, i : i + 1],
            )
        for i in range(4):
            nc.vector.tensor_tensor(
                prod[:], xn[:, i, :], wlab[:, i, :], op=mybir.AluOpType.mult
            )
            nc.vector.reduce_sum(
                dot[:, i : i + 1], prod[:], axis=mybir.AxisListType.X
            )
        snl = cpool.tile([128, 4], f32, tag="snl")
        nc.scalar.sqrt(snl[:], ssl[:])
        slinv = cpool.tile([128, 4], f32, tag="slinv")
        nc.vector.reciprocal(slinv[:], snl[:])
        cosl = cpool.tile([128, 4], f32, tag="cosl")
        nc.vector.tensor_tensor(cosl[:], dot[:], slinv[:], op=mybir.AluOpType.mult)
        # sine = sqrt(max(0, 1 - cos^2))
        sq = cpool.tile([128, 4], f32, tag="sq")
        nc.vector.tensor_tensor(sq[:], cosl[:], cosl[:], op=mybir.AluOpType.mult)
        sin2 = cpool.tile([128, 4], f32, tag="sin2")
        nc.vector.tensor_scalar(
            sin2[:], sq[:], -1.0, 1.0,
            op0=mybir.AluOpType.mult, op1=mybir.AluOpType.add,
        )
        nc.vector.tensor_scalar_max(sin2[:], sin2[:], 0.0)
        sine = cpool.tile([128, 4], f32, tag="sine")
        nc.scalar.sqrt(sine[:], sin2[:])
        # phi = cos*cos_m - sine*sin_m
        t1 = cpool.tile([128, 4], f32, tag="t1")
        nc.vector.tensor_scalar_mul(t1[:], cosl[:], COS_M)
        t2 = cpool.tile([128, 4], f32, tag="t2")
        nc.vector.tensor_scalar_mul(t2[:], sine[:], SIN_M)
        phi = cpool.tile([128, 4], f32, tag="phi")
        nc.vector.tensor_tensor(phi[:], t1[:], t2[:], op=mybir.AluOpType.subtract)
        # where(cos > th, phi, cos - mm), then * s
        alt = cpool.tile([128, 4], f32, tag="alt")
        nc.vector.tensor_scalar_sub(alt[:], cosl[:], MM)
        mask = cpool.tile([128, 4], mybir.dt.uint8, tag="mask")
        nc.vector.tensor_scalar(
            mask[:], cosl[:], TH, None, op0=mybir.AluOpType.is_gt,
        )
        phif = cpool.tile([128, 4], f32, tag="phif")
        nc.vector.tensor_copy(phif[:], alt[:])
        nc.vector.copy_predicated(phif[:], mask[:], phi[:])
        val = cpool.tile([128, 4], bf16, tag="val")
        nc.vector.tensor_scalar_mul(val[:], phif[:], S_SCALE)

        # ---- main loop over class-tile chunks
        chunk_sizes = [4] * 20 + [3]
        c0 = 0
        for n in chunk_sizes:
            wch = wpool.tile([128, n, D], f32, tag="wch")
            nc.sync.dma_start(out=wch[:], in_=w_view[:, c0 : c0 + n, :])
            ssw = spool.tile([128, 4], f32, tag="ssw")
            scrw = spool.tile([128, D], bf16, tag="scrw")
            for j in range(n):
                nc.scalar.activation(
                    out=scrw[:],
                    in_=wch[:, j, :],
                    func=mybir.ActivationFunctionType.Square,
                    accum_out=ssw[:, j : j + 1],
                )
            wn = spool.tile([128, 4], f32, tag="wn")
            nc.scalar.sqrt(wn[:, :n], ssw[:, :n])
            winv = spool.tile([128, 4], f32, tag="winv")
            nc.vector.reciprocal(winv[:, :n], wn[:, :n])
            wnb = wnbpool.tile([128, n, D], bf16, tag="wnb")
            for j in range(n):
                nc.vector.tensor_scalar_mul(
                    wnb[:, j, :], wch[:, j, :], winv[:, j : j + 1]
                )
            outch = outpool.tile([128, n, B], bf16, tag="outch")
            for g0 in range(0, n, 2):
                ng = min(2, n - g0)
                po = ppool_out.tile([128, 2 * B], f32, name="po")
                for jj in range(ng):
                    j = g0 + jj
                    pwt = ppool_wt.tile([128, D], bf16, name="pwt")
                    for k in range(4):
                        nc.tensor.transpose(
                            pwt[:, k * 128 : (k + 1) * 128],
                            wnb[:, j, k * 128 : (k + 1) * 128],
                            ident_bf[:],
                        )
                    wT = wtpool.tile([128, D], bf16, tag="wT")
                    nc.vector.tensor_copy(wT[:], pwt[:])
                    for k in range(4):
                        nc.tensor.matmul(
                            po[:, jj * B : (jj + 1) * B],
                            lhsT=wT[:, k * 128 : (k + 1) * 128],
                            rhs=xnT[k][:],
                            start=(k == 0),
                            stop=(k == 3),
                        )
                nc.vector.tensor_copy(
                    outch[:, g0 : g0 + ng, :], po[:, : ng * B]
                )
            nc.sync.dma_start(out=out_view[:, c0 : c0 + n, :], in_=outch[:])
            c0 += n

        # ---- scatter the 512 margin fixups into out
        for i in range(4):
            nc.gpsimd.indirect_dma_start(
                out=out_flat,
                out_offset=IndirectOffsetOnAxis(ap=soff_sb[:, i : i + 1], axis=0),
                in_=val[:, i : i + 1],
                in_offset=None,
            )

    nc.finalize()
    return nc


def _get_nc():
    if "nc" not in _CACHE:
        _CACHE["nc"] = _build_nc()
    return _CACHE["nc"]


def make_in_maps(x, weight, label):
    x = np.asarray(x, dtype=np.float32)
    weight = np.asarray(weight, dtype=np.float32)
    label = np.asarray(label).astype(np.int64)
    in_maps = []
    for i in range(NCORES):
        a, r = BASE[i], REAL[i]
        wshard = np.ones((CS, D), dtype=np.float32)
        wshard[:r] = weight[a : a + r]
        loc = label - a
        in_range = (loc >= 0) & (loc < r)
        idx = np.where(in_range, loc, PAD_ROW).astype(np.int32)
        b = np.arange(B, dtype=np.int64)
        soff = (idx.astype(np.int64) * B + b).astype(np.int32)
        # device layout [128, 4]: column i holds batch rows i*128..i*128+127
        idx_dev = np.ascontiguousarray(idx.reshape(4, 128).T)
        soff_dev = np.ascontiguousarray(soff.reshape(4, 128).T)
        in_maps.append({"x": x, "weight": wshard, "idx": idx_dev, "soff": soff_dev})
    return in_maps


def assemble(results):
    shards = [np.asarray(results[i]["out"])[: REAL[i]] for i in range(NCORES)]
    full_t = np.concatenate(shards, axis=0).astype(np.float32)  # [C, B]
    return np.ascontiguousarray(full_t.T)


def kernel(x, weight, label):
    from concourse.bass_utils import run_bass_kernel_spmd

    nc = _get_nc()
    in_maps = make_in_maps(x, weight, label)
    res = run_bass_kernel_spmd(nc, in_maps, list(range(NCORES)))
    return assemble(res.results)
